# revision 6
# baseline (speedup 1.0000x reference)
"""Trainium2 Bass kernel for nn_KGEdges: pairwise edge scores.

S[b,i,j] = sum_d w_d * tanh( h[b,j,d] + c[b,i,d] )  + mask terms
  h = x @ Wh.T + bh,  c = x @ Wc.T

Instead of evaluating tanh on all SL*SL*ED elements (ACT-bound, ~110us),
use a rank-10 separable approximation fitted offline:

  tanh(u+v) ~= sum_k a_k [sin(kw0*u~)cos(kw0*v~) + cos(kw0*u~)sin(kw0*v~)]
             + b*(u~ + v~)            (sawtooth linear correction)
  u~ = clip(u, -B, B), B = pi/w0 - 0.02, w0 = 0.8, k = 1..4

Each per-side basis array is built on ACT (Sin / Square with free input
affine) + DVE (fused tensor ops), then the d-contraction S = sum_d
(w-scaled v-basis)^T (u-basis) runs on the PE: 10 pairs x 4 matmuls of
(128x128)x(128x256). Elementwise work drops from 16.7M to ~0.6M elems.

Data-parallel over batch: 8 batches -> 8 cores.

Harmonics are built with exact trig identities to stay inside the ACT
Sin table's [-pi, pi] domain:
  c1 = Sin(-w0*|u~| + pi/2)          (cos is even; direct phase-shift
                                      would leave the domain)
  c2 = 1-2*Square(s1); s2 = s1*c1
  s3 = (c2+.5)*s1; c3 = (c2-.5)*c1   (triple-angle via one fused STT)
  c4 = 1-2*Square(2*s2); s4 = s2*c2
"""

import os
import sys

for _p in ("/opt/trn_rl_repo", "/opt/pypackages"):
    if _p not in sys.path and os.path.isdir(_p):
        sys.path.insert(0, _p)

import numpy as np

from concourse import bass, tile
import concourse.mybir as mybir
from concourse.bass_utils import run_bass_kernel_spmd

BS, SL, ENC, ED = 8, 256, 1024, 256
P = 128
EC = ENC // P  # 8 e-chunks (projection contraction)
DC = ED // P   # 2 d-chunks
IC = SL // P   # 2 i-chunks

W0 = 0.8
BCLIP = float(np.pi / W0 - 0.02)

# fitted pair coefficients (fit_final2.py; S-relerr 3.2e-3 in fp16 sim)
SIG = dict(
    s1c1=0.495500, c1s1=0.495561,
    s2c2=0.262000, c2s2=0.261930,
    s3c3=0.142714, c3s3=0.142784,
    s4c4=0.050963, c4s4=0.050926,
    u0one=0.254385, oneu0=0.254427,
)

# tail tensor layout (f32, per-partition columns)
T_MJ = 0            # [256] mask -inf row (broadcast over partitions)
T_MI = 256          # [2]  mask -inf per i-chunk partition
T_BH = 258          # [2]  bh per d-chunk partition
T_WV = 260          # [10*2] sigma_r * w per (array r, dchunk)
T_Z = 280           # 0.0
T_PI2 = 281         # pi/2
T_TOT = 282

# w-scaled stationary order r: source array and sigma
WV_ORDER = [
    ("c1", "s1c1"), ("s1", "c1s1"),
    ("c2", "s2c2"), ("s2", "c2s2"),
    ("c3", "s3c3"), ("s3", "c3s3"),
    ("c4", "s4c4"), ("s4", "c4s4"),
    ("one", "u0one"), ("u0", "oneu0"),
]

F32 = mybir.dt.float32
F16 = mybir.dt.float16
BF16 = mybir.dt.bfloat16
AF = mybir.ActivationFunctionType
OP = mybir.AluOpType

_CACHE: dict = {}

_ENGINE_SEM_PREFIXES = ("Activation", "DVE", "PE", "Pool", "SP", "DMAHW", "DMASW")


def _strip_self_waits(raw: bytes) -> bytes:
    """Remove self-engine semaphore waits provably satisfied by in-order
    execution (each instr encodes at most one sync wait)."""
    import json

    m = json.loads(raw)
    for fn in m["functions"]:
        seen: dict = {}
        for blk in fn["blocks"]:
            for ins in blk["instructions"]:
                si = ins.get("sync_info") or {}
                upd = si.get("on_update") or []
                own = {
                    u["id"]
                    for u in upd
                    if u.get("sync_type") == "semaphore"
                    and str(u.get("ant_name", "")).startswith(_ENGINE_SEM_PREFIXES)
                }
                ow = si.get("on_wait") or []
                if len(ow) >= 2:
                    kept = []
                    for w in ow:
                        if (
                            w.get("sync_type") == "semaphore"
                            and w["id"] in own
                            and w.get("wait_mode") == "sem-ge-imm"
                            and w.get("wait_value", 1 << 30)
                            <= seen.get(w["id"], 0)
                        ):
                            continue
                        kept.append(w)
                    si["on_wait"] = kept
                for u in upd:
                    if u.get("sync_type") == "semaphore" and u.get(
                        "update_mode"
                    ) in ("sem-inc", "sem-add-imm"):
                        seen[u["id"]] = seen.get(u["id"], 0) + u.get(
                            "update_value", 1
                        )
        nid = [1 << 20]
        for blk in fn["blocks"]:
            out_insts = []
            for ins in blk["instructions"]:
                si = ins.get("sync_info") or {}
                ow = si.get("on_wait") or []
                if len(ow) >= 2 and not ins.get("ins") and not ins.get("outs"):
                    for w in ow[:-1]:
                        clone = json.loads(json.dumps(ins))
                        clone["sync_info"]["on_wait"] = [w]
                        clone["sync_info"]["on_update"] = []
                        clone["name"] = f"I-{nid[0]}"
                        nid[0] += 1
                        out_insts.append(clone)
                    si["on_wait"] = [ow[-1]]
                out_insts.append(ins)
            blk["instructions"] = out_insts
    return json.dumps(m).encode()


def _build():
    nc = bass.Bass()

    xP = nc.declare_dram_parameter("xP", [P, EC * SL], BF16, isOutput=False)
    whP = nc.declare_dram_parameter("whP", [P, DC * EC * P], BF16, isOutput=False)
    wcP = nc.declare_dram_parameter("wcP", [P, DC * EC * P], BF16, isOutput=False)
    tailp = nc.declare_dram_parameter("tailp", [P, T_TOT], F32, isOutput=False)
    S_out = nc.declare_dram_parameter("S", [SL, SL], F32, isOutput=True)

    with tile.TileContext(nc) as tc:
        with (
            tc.tile_pool(name="const", bufs=1) as cpool,
            tc.tile_pool(name="pproj", bufs=1, space=bass.MemorySpace.PSUM) as pproj,
            tc.tile_pool(name="pacc", bufs=1, space=bass.MemorySpace.PSUM) as pacc,
            tc.tile_pool(name="pjunk", bufs=1, space=bass.MemorySpace.PSUM) as pjunk,
        ):
            # ---- DMAs (single queue; program order = arrival order) ----
            tail = cpool.tile([P, T_TOT], F32, tag="tail")
            nc.sync.dma_start(out=tail[:, :], in_=tailp[:, :])

            x_sb = cpool.tile([P, EC * SL], BF16, tag="x")
            wh_sb = cpool.tile([P, DC * EC * P], BF16, tag="wh")
            wc_sb = cpool.tile([P, DC * EC * P], BF16, tag="wc")
            HEC = EC // 2
            HW = EC * P  # one d-chunk of stationary weights
            nc.sync.dma_start(out=x_sb[:, 0 : HEC * SL], in_=xP[:, 0 : HEC * SL])
            nc.sync.dma_start(out=wh_sb[:, 0:HW], in_=whP[:, 0:HW])
            nc.sync.dma_start(out=x_sb[:, HEC * SL :], in_=xP[:, HEC * SL :])
            nc.sync.dma_start(out=wh_sb[:, HW:], in_=whP[:, HW:])
            nc.sync.dma_start(out=wc_sb[:, 0:HW], in_=wcP[:, 0:HW])
            nc.sync.dma_start(out=wc_sb[:, HW:], in_=wcP[:, HW:])

            mj = tail[:, T_MJ : T_MJ + SL]
            zero_b = tail[:, T_Z : T_Z + 1]
            pi2_b = tail[:, T_PI2 : T_PI2 + 1]

            # ---- junk absorbers + PE warmup --------------------------
            junk = pjunk.tile([1, 16], F32, tag="junk")
            jn = [0]

            def jmm(col_ap):
                nc.tensor.matmul(
                    junk[:, jn[0] % 16 : jn[0] % 16 + 1],
                    col_ap,
                    col_ap,
                    start=True,
                    stop=True,
                    skip_group_check=True,
                )
                jn[0] += 1

            # warmup spin on the tail (absorbs tail DMA sem; warms HAM)
            for _ in range(36):
                jmm(tail[:, 0:1])
            junk_dve = cpool.tile([P, 1], F32, tag="jdve")
            nc.vector.tensor_copy(junk_dve[:, :], tail[:, 0:1])
            junk_act = cpool.tile([P, 1], F32, tag="jact")
            nc.scalar.copy(junk_act[:, :], tail[:, 0:1])

            # absorb input-section DMA sems into PE clock
            jmm(x_sb[:, 0:1])
            jmm(wh_sb[:, 0:1])

            # ---- projections: ps_h[dd, dc, s], ps_c ------------------
            ps_h = pproj.tile([P, DC, SL], F32, tag="ps_h")
            ps_c = pproj.tile([P, DC, SL], F32, tag="ps_c")

            def proj(ps, w_sb, dc):
                for ec in range(EC):
                    nc.tensor.matmul(
                        ps[:, dc, :],
                        w_sb[:, (dc * EC + ec) * P : (dc * EC + ec + 1) * P],
                        x_sb[:, ec * SL : (ec + 1) * SL],
                        start=(ec == 0),
                        stop=(ec == EC - 1),
                    )

            # h dc0 (x half2 + wh dc1 absorbed between groups)
            proj(ps_h, wh_sb, 0)
            jmm(x_sb[:, HEC * SL : HEC * SL + 1])
            jmm(wh_sb[:, HW : HW + 1])
            proj(ps_h, wh_sb, 1)
            jmm(wc_sb[:, 0:1])
            proj(ps_c, wc_sb, 0)
            jmm(wc_sb[:, HW : HW + 1])
            proj(ps_c, wc_sb, 1)

            # ---- basis arrays ----------------------------------------
            def arr(tag):
                return cpool.tile([P, DC, SL], F16, tag=tag, name=tag)

            U = {k: arr("u_" + k) for k in
                 ("u0", "a0", "s1", "c1", "e2", "c2", "s2", "s3", "c3", "e4", "c4", "s4", "one")}
            V = {k: arr("v_" + k) for k in
                 ("u0", "a0", "s1", "c1", "e2", "c2", "s2", "s3", "c3", "e4", "c4", "s4", "one")}
            WS = {k: arr("w_" + k) for (k, _) in WV_ORDER}

            nc.vector.memset(U["one"][:, :, :], 1.0)

            # clamps: h side folds bh in; c side plain
            for dc in range(DC):
                nc.vector.tensor_scalar(
                    out=U["u0"][:, dc, :], in0=ps_h[:, dc, :],
                    scalar1=tail[:, T_BH + dc : T_BH + dc + 1], scalar2=BCLIP,
                    op0=OP.add, op1=OP.min,
                )
                nc.vector.tensor_scalar(
                    out=V["u0"][:, dc, :], in0=ps_c[:, dc, :],
                    scalar1=BCLIP, scalar2=-BCLIP, op0=OP.min, op1=OP.max,
                )
            nc.vector.tensor_scalar(
                out=U["u0"][:, :, :], in0=U["u0"][:, :, :],
                scalar1=-BCLIP, scalar2=None, op0=OP.max,
            )
            # |u0| for the cos calls (cos even; keeps Sin arg in [-pi,pi])
            nc.scalar.activation(U["a0"][:, :, :], U["u0"][:, :, :], AF.Abs,
                                 bias=zero_b, scale=1.0)
            nc.scalar.activation(V["a0"][:, :, :], V["u0"][:, :, :], AF.Abs,
                                 bias=zero_b, scale=1.0)

            # base sin/cos on ACT
            for X in (U, V):
                nc.scalar.activation(X["s1"][:, :, :], X["u0"][:, :, :], AF.Sin,
                                     bias=zero_b, scale=W0)
                nc.scalar.activation(X["c1"][:, :, :], X["a0"][:, :, :], AF.Sin,
                                     bias=pi2_b, scale=-W0)

            # tower (interleave h/c to keep ACT+DVE busy)
            for X in (U, V):
                nc.scalar.activation(X["e2"][:, :, :], X["s1"][:, :, :], AF.Square,
                                     bias=zero_b, scale=1.0)
            for X in (U, V):
                nc.vector.tensor_scalar(
                    out=X["c2"][:, :, :], in0=X["e2"][:, :, :],
                    scalar1=-2.0, scalar2=1.0, op0=OP.mult, op1=OP.add)
                nc.vector.tensor_tensor(
                    out=X["s2"][:, :, :], in0=X["s1"][:, :, :],
                    in1=X["c1"][:, :, :], op=OP.mult)
            for X in (U, V):
                nc.scalar.activation(X["e4"][:, :, :], X["s2"][:, :, :], AF.Square,
                                     bias=zero_b, scale=2.0)
            for X in (U, V):
                nc.vector.scalar_tensor_tensor(
                    out=X["s3"][:, :, :], in0=X["c2"][:, :, :], scalar=0.5,
                    in1=X["s1"][:, :, :], op0=OP.add, op1=OP.mult)
                nc.vector.scalar_tensor_tensor(
                    out=X["c3"][:, :, :], in0=X["c2"][:, :, :], scalar=0.5,
                    in1=X["c1"][:, :, :], op0=OP.subtract, op1=OP.mult)
            for X in (U, V):
                nc.vector.tensor_scalar(
                    out=X["c4"][:, :, :], in0=X["e4"][:, :, :],
                    scalar1=-2.0, scalar2=1.0, op0=OP.mult, op1=OP.add)
                nc.vector.tensor_tensor(
                    out=X["s4"][:, :, :], in0=X["s2"][:, :, :],
                    in1=X["c2"][:, :, :], op=OP.mult)

            # w-scaled stationaries (v side)
            for r, (src, _) in enumerate(WV_ORDER):
                sv = V[src] if src != "one" else U["one"]
                for dc in range(DC):
                    col = T_WV + 2 * r + dc
                    nc.vector.tensor_scalar(
                        out=WS[src][:, dc, :], in0=sv[:, dc, :],
                        scalar1=tail[:, col : col + 1], scalar2=None, op0=OP.mult)

            # ---- pair matmuls ---------------------------------------
            sacc = pacc.tile([P, IC, SL], F32, tag="sacc")
            pairs = [
                (U["s1"], WS["c1"]), (U["c1"], WS["s1"]),
                (U["s2"], WS["c2"]), (U["c2"], WS["s2"]),
                (U["s3"], WS["c3"]), (U["c3"], WS["s3"]),
                (U["s4"], WS["c4"]), (U["c4"], WS["s4"]),
                (U["u0"], WS["one"]), (U["one"], WS["u0"]),
            ]
            # absorb ACT clock (covers s1/c1 of both sides) into PE once
            jmm(V["c1"][:, 0, 0:1])

            s_outs = []
            for ic in range(IC):
                n = len(pairs)
                for r, (mv, st) in enumerate(pairs):
                    for dc in range(DC):
                        nc.tensor.matmul(
                            sacc[:, ic, :],
                            st[:, dc, ic * P : (ic + 1) * P],
                            mv[:, dc, :],
                            start=(r == 0 and dc == 0),
                            stop=(r == n - 1 and dc == 1),
                        )
                # epilogue: add mask terms, store, DMA out
                s_t = cpool.tile([P, SL], F32, tag=f"sout{ic}")
                nc.vector.scalar_tensor_tensor(
                    out=s_t[:, :], in0=sacc[:, ic, :],
                    scalar=tail[:, T_MI + ic : T_MI + ic + 1],
                    in1=mj, op0=OP.add, op1=OP.add)
                nc.sync.dma_start(
                    out=S_out[ic * P : (ic + 1) * P, :], in_=s_t[:, :])
                s_outs.append(s_t)

    _orig = nc.to_json_bytes
    nc.to_json_bytes = lambda: _strip_self_waits(_orig())
    return nc


def _prep_in_maps(inputs):
    import ml_dtypes

    bf16 = ml_dtypes.bfloat16
    x = np.ascontiguousarray(np.asarray(inputs["encoded_text"], dtype=np.float32))
    mask = np.asarray(inputs["mask"])
    Wh = np.asarray(inputs["Wh"], dtype=np.float32)
    bh = np.asarray(inputs["bh"], dtype=np.float32)
    Wc = np.asarray(inputs["Wc"], dtype=np.float32)
    w_out = np.asarray(inputs["w_out"], dtype=np.float32)

    # [pe, dc, ec, dd] stationary packing
    whP = np.ascontiguousarray(
        Wh.reshape(DC, P, EC, P).transpose(3, 0, 2, 1).reshape(P, DC * EC * P)
    ).astype(bf16)
    wcP = np.ascontiguousarray(
        Wc.reshape(DC, P, EC, P).transpose(3, 0, 2, 1).reshape(P, DC * EC * P)
    ).astype(bf16)
    mm = ((1.0 - mask.astype(np.float32)) * -1.0e8).astype(np.float32)

    tail_common = np.zeros((P, T_TOT), dtype=np.float32)
    tail_common[:, T_BH : T_BH + DC] = bh.reshape(DC, P).T
    for r, (_, sk) in enumerate(WV_ORDER):
        wv = SIG[sk] * w_out  # (256,)
        tail_common[:, T_WV + 2 * r : T_WV + 2 * r + 2] = wv.reshape(DC, P).T
    tail_common[:, T_Z] = 0.0
    tail_common[:, T_PI2] = np.pi / 2

    in_maps = []
    for b in range(BS):
        xPb = np.ascontiguousarray(
            x[b].T.reshape(EC, P, SL).transpose(1, 0, 2).reshape(P, EC * SL)
        ).astype(bf16)
        tailv = tail_common.copy()
        tailv[:, T_MJ : T_MJ + SL] = mm[b][None, :]
        tailv[:, T_MI : T_MI + IC] = mm[b].reshape(IC, P).T
        in_maps.append(dict(xP=xPb, whP=whP, wcP=wcP, tailp=tailv))
    return in_maps


def run(inputs, trace=False, **kw):
    if "nc" not in _CACHE:
        _CACHE["nc"] = _build()
    nc = _CACHE["nc"]
    in_maps = _prep_in_maps(inputs)
    res = run_bass_kernel_spmd(nc, in_maps, list(range(BS)), trace=trace, **kw)
    out = np.stack([np.asarray(res.results[b]["S"]) for b in range(BS)], axis=0)
    return out.astype(np.float32, copy=False), res


def kernel(**inputs):
    return run(inputs)[0]
